# revision 53
# baseline (speedup 1.0000x reference)
"""Trainium2 Bass kernel for a GPT-style decoder block (B=2, T=2048, C=768, H=12).

Sharding: 8 cores = 2 batches x 4 interleaved block-sets. Core c owns 128-row
blocks {c, c+4, c+8, c+12} of its batch. Its context buffer holds the 16
position-blocks [zeros x (3-c) | blocks 0..12+c]; the own blocks then sit at
the STATIC positions {3, 7, 11, 15} with causal context = position prefixes of
length {4, 8, 12, 16} blocks. Every core runs the same instruction stream.

v2 restructure (vs the LN-on-chip baseline):
- LN1 statistics, the normalize, the x^T transpose and the fp8 quantization
  all happen on the HOST (numpy) inside kernel(): the device receives
  xnT8 (pre-transposed fp8*16 context, zero-padded), xnT8own (own rows) and
  xn_own (f32 residual base = ln1(x)*g1 + b1r, own rows). This removes 96 PE
  transposes, all bn_stats/normalize work and the 30us DMA-behind-weights
  startup stall from the critical path.
- DMA order: wkown (xnT8own|wk merged, one transfer) -> packed f32 consts
  -> wq -> xnT8 -> wv -> packed bf16 consts; every separate transfer pays
  ~1.5us queue-head latency at kernel start, so small consts are packed
  into two tensors and the K-projection inputs are merged into one.
  xn_own / w2 / w1 stream later, under attention.
- Projections: K (wk stationary), Q (wq stationary, nb-inner with 4 PSUM
  banks so each DoubleRow LDWEIGHTS serves 4 N=512 matmuls), V (xnT8 tile
  stationary, j-outer g-inner so each LDWEIGHTS serves 2 matmuls).
- Attention: psa/psb double-buffered score tiles; the exp for 4 of 6 head
  pairs' first head runs on the Vector engine via a one-instruction
  Schraudolph approximation (bf16 bits of exp(x/8) == int16(A*x + B),
  uint16 output so masked negatives saturate to 0; the softmax num/den
  correlation cancels the systematic bias), balancing the exp load across
  ACT and DVE; causal tri-mask accumulated into PSUM by a PE matmul
  (tri = -704 so masked entries stay in the Schraudolph int16 window);
  per-head y/den normalization + residual accumulation run in attention's
  shadow; small warm-keeper matmuls hold the PE HAM clock at 8/8 through
  the LN2 stretch.
- MLP: W1 DMA'd into the post-attention SBUF space in 6 slice transfers
  at LN2 time (attention pools are closed by then, so the full 36KB fits);
  W1 computed in two own-row halves so the first half starts right after
  LN2 slots 0/1 transpose; W2 fp8 DoubleRow as before.

Numerics vs the fixed-seed reference: ~1.75e-2 measured on HW (gate 2e-2):
fp8e4 QKV & W2 (DoubleRow), bf16 scores/P/V and W1, fp32 LN statistics
(host, f64), softmax normalization, residuals and output.
"""

import os

import numpy as np

B, T, C = 2, 2048, 768
H, DH = 12, 64
F = 4 * C
R = 512            # own rows per core
NT = 16            # ctx position blocks
NC = C // 128      # 6
JC = NC // 2       # 3 channel pairs
NF = F // 128      # 24
JF = NF // 2       # 12 hidden pairs
HP = H // 2        # 6 head pairs
VS = 66            # per-head stride in Vt (64 v + 1 ones + pad)
EPS = 1e-3
SX = 16.0          # fp8 scale on normalized activations
SW = 512.0         # fp8 scale on weights
SXW = SX * SW      # 8192

_CACHE = {}


def _build_program():
    import concourse.bass as bass  # noqa: F401
    import concourse.mybir as mybir
    import concourse.tile as tile
    from concourse import bacc

    dt = mybir.dt
    f32 = dt.float32
    bf16 = dt.bfloat16
    fp8 = dt.float8e4
    AF = mybir.ActivationFunctionType
    ALU = mybir.AluOpType
    PM = mybir.MatmulPerfMode

    nc = bacc.Bacc("TRN2", target_bir_lowering=False, debug=False, num_devices=8)

    # ---- DRAM I/O ----
    # all small consts packed into two tensors: each separate DMA pays
    # ~1.5us of queue-head latency at kernel start, so 7 transfers -> 2.
    # constsf f32 [128, 36]: bq | bk | b1 (column-major per-hp/nf biases)
    # constsb bf16 [128, 1216]: tri | ident | vones(16*12) | b2row(row 0)
    cf_d = nc.dram_tensor("constsf", [128, 2 * HP + NF], f32,
                          kind="ExternalInput")
    cb_d = nc.dram_tensor("constsb", [128, 256 + NT * H + C], bf16,
                          kind="ExternalInput")
    # xnT8own and wk merged into one tensor/DMA (queue-head latency):
    # cols 0:R = own rows, R:R+C = wk
    ko_d = nc.dram_tensor("wkown", [128, JC, 2, R + C], fp8,
                          kind="ExternalInput")
    xt8_d = nc.dram_tensor("xnT8", [128, JC, 2, T], fp8, kind="ExternalInput")
    wq_d = nc.dram_tensor("wq8", [128, JC, 2, C], fp8, kind="ExternalInput")
    wv_d = nc.dram_tensor("wv8", [128, JC, 2, C], fp8, kind="ExternalInput")
    xn_own_d = nc.dram_tensor("xn_own", [128, 4, C], f32, kind="ExternalInput")
    w2_d = nc.dram_tensor("w28", [128, JF, 2, C], fp8, kind="ExternalInput")
    w1_d = nc.dram_tensor("w1b", [128, NC, F], bf16, kind="ExternalInput")
    # output stays partition-major (matches SBUF) so the final DMAs are
    # fully contiguous; the host gather unscrambles slots
    out_d = nc.dram_tensor("out", [128, 4, C], f32, kind="ExternalOutput")

    with tile.TileContext(nc) as tc:
        with (
            tc.tile_pool(name="const", bufs=1) as constp,
            tc.tile_pool(name="keep", bufs=1) as keepp,
            tc.tile_pool(name="w2pool", bufs=1) as w2p,
            tc.tile_pool(name="stat2", bufs=1) as stat2p,
        ):
            cF = constp.tile([128, 2 * HP + NF], f32)
            cB = constp.tile([128, 256 + NT * H + C], bf16)
            ones1 = constp.tile([1, 128], bf16)
            nc.vector.memset(ones1[:], 1.0)
            eps_t = constp.tile([128, 1], f32)
            nc.vector.memset(eps_t[:], EPS)

            x1 = keepp.tile([128, 4, C], f32)        # residual base, then +y
            yrow = keepp.tile([128, 4, C], f32)      # y/den, token-major
            y_sb = keepp.tile([128, 4, H, 65], bf16)  # y token-major; k=3-s

            with (
                tc.tile_pool(name="xnT8", bufs=1) as xnT8p,
                tc.tile_pool(name="QT", bufs=1) as QTp,
                tc.tile_pool(name="KT", bufs=1) as KTp,
                tc.tile_pool(name="V", bufs=1) as Vp,
            ):
                wko = KTp.tile([128, JC, 2, R + C], fp8)
                nc.sync.dma_start(wko[:], ko_d[:])
                KT = KTp.tile([128, HP, R], bf16)
                QT = QTp.tile([128, HP, T], bf16)
                Vt = Vp.tile([128, NT, H, VS], bf16)
                xnT8 = xnT8p.tile([128, JC, 2, T], fp8)

                # ===== Phase A: QKV projections =====
                with (
                    tc.tile_pool(name="wqkv", bufs=1) as wp,
                    tc.tile_pool(name="psQ", bufs=1, space="PSUM") as psQ,
                    tc.tile_pool(name="psV", bufs=2, space="PSUM") as psV,
                ):
                    nc.sync.dma_start(cF[:], cf_d[:])
                    wq = wp.tile([128, JC, 2, C], fp8, name="wq8")
                    nc.sync.dma_start(wq[:], wq_d[:])
                    nc.sync.dma_start(xnT8[:], xt8_d[:])
                    wv = wp.tile([128, JC, 2, C], fp8, name="wv8")
                    nc.sync.dma_start(wv[:], wv_d[:])
                    nc.sync.dma_start(cB[:], cb_d[:])
                    # ones column of Vt (zero for padding blocks)
                    nc.vector.tensor_copy(
                        Vt[:, :, :, 64],
                        cB[:, 256:256 + NT * H].rearrange(
                            "p (t h) -> p t h", t=NT),
                    )

                    # K projection (own rows): wk stationary per (hp, j)
                    for hp in range(HP):
                        ps = psQ.tile([128, 512], f32, tag="q%d" % (hp % 4),
                                      name="psk")
                        for j in range(JC):
                            nc.tensor.matmul(
                                ps[:],
                                wko[:, j, :, R + hp * 128:R + (hp + 1) * 128],
                                wko[:, j, :, 0:R],
                                start=(j == 0), stop=(j == JC - 1),
                                perf_mode=PM.DoubleRow,
                            )
                        if hp % 2 == 0:
                            nc.vector.tensor_scalar(
                                KT[:, hp, :], ps[:], 1.0 / SXW, cF[:, HP + hp:HP + hp + 1],
                                op0=ALU.mult, op1=ALU.add,
                            )
                        else:
                            nc.scalar.activation(
                                KT[:, hp, :], ps[:], AF.Identity,
                                bias=cF[:, HP + hp:HP + hp + 1], scale=1.0 / SXW,
                            )

                    # Q projection: wq stationary per (hp, j) serves 4 matmuls
                    for hp in range(HP):
                        pss = [psQ.tile([128, 512], f32, tag="q%d" % nb,
                                        name="psq%d" % nb) for nb in range(4)]
                        for j in range(JC):
                            for nb in range(4):
                                nc.tensor.matmul(
                                    pss[nb][:],
                                    wq[:, j, :, hp * 128:(hp + 1) * 128],
                                    xnT8[:, j, :, nb * 512:(nb + 1) * 512],
                                    start=(j == 0), stop=(j == JC - 1),
                                    perf_mode=PM.DoubleRow,
                                )
                        for nb in range(4):
                            if (hp + nb) % 2 == 0:
                                nc.scalar.activation(
                                    QT[:, hp, nb * 512:(nb + 1) * 512],
                                    pss[nb][:], AF.Identity,
                                    bias=cF[:, hp:hp + 1], scale=1.0 / SXW,
                                )
                            else:
                                nc.vector.tensor_scalar(
                                    QT[:, hp, nb * 512:(nb + 1) * 512],
                                    pss[nb][:], 1.0 / SXW, cF[:, hp:hp + 1],
                                    op0=ALU.mult, op1=ALU.add,
                                )


                    # V projection: xnT8 tile stationary per (tb, j)
                    for tb in range(NT):
                        ps = psV.tile([128, 2, 8, 64], f32, tag="psV", name="psv")
                        for j in range(JC):
                            for g in range(2):
                                nc.tensor.matmul(
                                    ps[:, g, 0:6, :],
                                    xnT8[:, j, :, tb * 128:(tb + 1) * 128],
                                    wv[:, j, :, g * 384:(g + 1) * 384],
                                    start=(j == 0), stop=(j == JC - 1),
                                    perf_mode=PM.DoubleRow,
                                )
                        for g in range(2):
                            if (tb + g) % 2 == 0:
                                nc.vector.tensor_scalar(
                                    Vt[:, tb, g * 6:(g + 1) * 6, 0:64],
                                    ps[:, g, 0:6, :], 1.0 / SXW, None,
                                    op0=ALU.mult,
                                )
                            else:
                                nc.scalar.activation(
                                    Vt[:, tb, g * 6:(g + 1) * 6, 0:64],
                                    ps[:, g, 0:6, :],
                                    AF.Identity, scale=1.0 / SXW,
                                )

                # residual base + deferred weights (DMA after the QKV weights)
                nc.sync.dma_start(x1[:], xn_own_d[:])
                w2 = w2p.tile([128, JF, 2, C], fp8, name="w28")
                nc.sync.dma_start(w2[:], w2_d[:])

                # ===== Phase C: attention =====
                # (st6 lives here so the g=0 stats can run in C's shadow)
                # Schraudolph exp on DVE for head A: the bf16 bit pattern of
                # exp(x/8) == int16(AEXP*x + BEXP) (the /65536 folds the
                # >>16 into the mac; the int16 convert rounds, ~0.8% noise
                # inside Schraudolph's 3.5% band; num/den correlation cancels
                # the systematic bias in the softmax). tri = -704 ~=
                # -B/A makes masked entries land near zero, so the int16 is
                # a ~2^-117 bf16 denormal (effectively zero) without a
                # clamp, while exp(-88) == 0 on the ACT path too.
                AEXP = float(2.0 ** 23 / np.log(2.0) * 0.125 / 65536.0)
                BEXP = float((127 * 2 ** 23 - 366000) / 65536.0)
                u16 = dt.uint16  # f32->uint16 saturates negatives to 0
                with (
                    tc.tile_pool(name="exps", bufs=2) as expp,
                    tc.tile_pool(name="yT", bufs=2) as ytp,
                    tc.tile_pool(name="yn", bufs=2) as ynp,
                    tc.tile_pool(name="psS", bufs=2, space="PSUM") as psS,
                    tc.tile_pool(name="psY", bufs=2, space="PSUM") as psY,
                ):
                    def emit_scores_pair(hp, expAB):
                        # one step per ctx pair: psa = head 2hp, psb = head
                        # 2hp+1; head A exp on DVE, head B exp on ACT
                        for jp in range(NT // 2):
                            Np = (4 - jp // 2) * 128
                            diag = (jp % 2 == 1)  # P=2jp+1 is a diag block
                            psa = psS.tile([128, 2, 512], f32, tag="psS",
                                           name="pssa")
                            psb = psS.tile([128, 2, 512], f32, tag="psS",
                                           name="pssb")
                            for ql in range(2):
                                P = 2 * jp + ql
                                dq = diag and ql == 1
                                for z, ps in ((0, psa), (1, psb)):
                                    nc.tensor.matmul(
                                        ps[:, ql, 0:Np],
                                        QT[64 * z:64 * z + 64, hp,
                                           P * 128:(P + 1) * 128],
                                        KT[64 * z:64 * z + 64, hp, 0:Np],
                                        start=True, stop=not dq,
                                        skip_group_check=dq,
                                    )
                            if diag:  # accumulate tri into the diag slice
                                for ps in (psa, psb):
                                    nc.tensor.matmul(
                                        ps[:, 1, Np - 128:Np],
                                        cB[:, 128:256], cB[:, 0:128],
                                        start=False, stop=True,
                                        skip_group_check=True,
                                    )
                            if hp != 1 and hp != 4:
                                nc.vector.tensor_scalar(
                                    expAB[:, 0, 2 * jp:2 * jp + 2, 0:Np].bitcast(u16),
                                    psa[:, :, 0:Np], AEXP, BEXP,
                                    op0=ALU.mult, op1=ALU.add,
                                )
                            else:
                                nc.scalar.activation(
                                    expAB[:, 0, 2 * jp:2 * jp + 2, 0:Np],
                                    psa[:, :, 0:Np], AF.Exp, scale=0.125,
                                )
                            nc.scalar.activation(
                                expAB[:, 1, 2 * jp:2 * jp + 2, 0:Np],
                                psb[:, :, 0:Np], AF.Exp, scale=0.125,
                            )
                            yield

                    def emit_pv(h, z, expAB):
                        # generator: one step per ctx pair (2 PV matmuls)
                        psy = psY.tile([128, 512], f32, tag="psY", name="psy")
                        for jp in range(NT // 2):
                            for ql in range(2):
                                P = 2 * jp + ql
                                Np = (4 - P // 4) * 128
                                nc.tensor.matmul(
                                    psy[0:65, 0:Np],
                                    Vt[:, P, h, 0:65],
                                    expAB[:, z, P, 0:Np],
                                    start=(P == 0), stop=(P == NT - 1),
                                    skip_group_check=True,
                                )
                            yield
                        yTb = ytp.tile([128, 512], bf16, tag="yT", name="yT")
                        nc.vector.tensor_copy(yTb[0:65, :], psy[0:65, :])
                        tpy = psY.tile([128, 4, 66], bf16, tag="psTy", name="tpy")
                        for k in range(4):
                            nc.tensor.matmul(
                                tpy[:, k, 0:65], yTb[0:65, k * 128:(k + 1) * 128],
                                cB[0:65, 128:193], is_transpose=True,
                                start=True, stop=True,
                            )
                        if h % 2 == 0:
                            nc.scalar.copy(y_sb[:, :, h, :], tpy[:, :, 0:65])
                        else:
                            nc.vector.tensor_copy(y_sb[:, :, h, :], tpy[:, :, 0:65])
                        yield
                        # y/den into yrow (runs in attention's shadow)
                        den = ynp.tile([128, 4], f32, tag="den", name="den")
                        nc.vector.tensor_copy(den[:], y_sb[:, :, h, 64])
                        rec = ynp.tile([128, 4], f32, tag="rec", name="rec")
                        nc.vector.reciprocal(rec[:], den[:])
                        for s in range(4):
                            k = 3 - s
                            if (h + s) % 2 == 0:
                                nc.vector.tensor_scalar(
                                    yrow[:, s, h * 64:(h + 1) * 64],
                                    y_sb[:, k, h, 0:64],
                                    rec[:, k:k + 1], None, op0=ALU.mult,
                                )
                            else:
                                nc.scalar.activation(
                                    yrow[:, s, h * 64:(h + 1) * 64],
                                    y_sb[:, k, h, 0:64],
                                    AF.Identity, scale=rec[:, k:k + 1],
                                )
                        yield

                    # software pipeline: pair i scores/exp woven with the
                    # previous pair's two PV streams
                    prev_pvs = []
                    for hp2 in range(HP):
                        expAB = expp.tile([128, 2, NT, 512], bf16,
                                          tag="expST", name="expAB")
                        sc = emit_scores_pair(hp2, expAB)
                        for _ in sc:
                            for pv in prev_pvs:
                                next(pv, None)
                        for pv in prev_pvs:  # drain tails
                            for _ in pv:
                                pass
                        prev_pvs = [emit_pv(2 * hp2, 0, expAB),
                                    emit_pv(2 * hp2 + 1, 1, expAB)]
                    # heads 0-9 are final: fold them into the residual and
                    # take the g=0 LN2 stats now, then drain the last PV
                    # pair -- its matmuls keep the PE HAM window alive while
                    # this DVE-serial stretch runs, so the MLP starts warm
                    st6s = [stat2p.tile([128, 3, 6], f32, tag="st6%d" % s,
                                        name="st6b") for s in range(4)]
                    for s in range(4):
                        nc.vector.tensor_add(
                            x1[:, s, 0:384], x1[:, s, 0:384], yrow[:, s, 0:384])
                        nc.vector.tensor_add(
                            x1[:, s, 384:640], x1[:, s, 384:640],
                            yrow[:, s, 384:640])
                    for s in range(4):
                        nc.vector.bn_stats(st6s[s][:, 0, :], x1[:, s, 0:384])
                        nc.vector.bn_stats(st6s[s][:, 1, :], x1[:, s, 384:640])
                    for pv in prev_pvs:
                        for _ in pv:
                            pass

            # ===== Phase D: +y, LN2 =====
            with (
                tc.tile_pool(name="x1nT", bufs=1) as x1nTp,
                tc.tile_pool(name="h1T8", bufs=1) as h1p,
                tc.tile_pool(name="w1res", bufs=1) as w1rp,
                tc.tile_pool(name="x1nbf", bufs=1) as x1nbfp,
                tc.tile_pool(name="psT2", bufs=2, space="PSUM") as psT2,
            ):
                x1nT = x1nTp.tile([128, NC, R], bf16)
                h1T8 = h1p.tile([128, JF, 2, R], fp8)
                w1r = w1rp.tile([128, NC, F], bf16, name="w1b")
                for i in range(6):
                    nc.sync.dma_start(w1r[:, :, i * 512:(i + 1) * 512],
                                      w1_d[:, :, i * 512:(i + 1) * 512])

                warm = psT2.tile([128, 128], f32, tag="warm", name="warm")
                st2s, rstds, nmbs, x1ns = [], [], [], []
                for s in range(4):
                    nc.vector.tensor_add(
                        x1[:, s, 640:768], x1[:, s, 640:768],
                        yrow[:, s, 640:768])
                    nc.vector.bn_stats(st6s[s][:, 2, :], x1[:, s, 640:768])
                for s in range(4):
                    st2 = stat2p.tile([128, 2], f32, tag="st2%d" % s, name="st2b")
                    nc.vector.bn_aggr(st2[:], st6s[s][:])
                    st2s.append(st2)
                    std = stat2p.tile([128, 1], f32, tag="std%d" % s, name="stdb")
                    nc.scalar.activation(std[:], st2[:, 1:2], AF.Sqrt,
                                         bias=eps_t[:])
                    rstd = stat2p.tile([128, 1], f32, tag="rstd%d" % s,
                                       name="rstdb")
                    nc.vector.reciprocal(rstd[:], std[:])
                    rstds.append(rstd)
                for s in range(4):
                    nc.tensor.matmul(
                        warm[0:2, 0:128], st2s[s][:], x1[:, s, 0:128],
                        start=True, stop=True, skip_group_check=True,
                    )
                    nmb = stat2p.tile([128, 1], f32, tag="nmb%d" % s, name="nmbb")
                    nc.vector.tensor_scalar(
                        nmb[:], st2s[s][:, 0:1], rstds[s][:], -1.0,
                        op0=ALU.mult, op1=ALU.mult,
                    )
                    x1n = x1nbfp.tile([128, C], bf16, tag="x1n%d" % s, name="x1n")
                    nc.scalar.activation(
                        x1n[:], x1[:, s, :], AF.Identity, bias=nmb[:],
                        scale=rstds[s][:]
                    )
                    x1ns.append(x1n)
                def emit_transpose(s):
                    tp = psT2.tile([128, NC, 128], bf16, tag="psT2", name="tpb")
                    for cb in range(NC):
                        nc.tensor.matmul(
                            tp[:, cb, :],
                            x1ns[s][:, cb * 128:(cb + 1) * 128],
                            cB[:, 128:256], is_transpose=True, start=True, stop=True,
                        )
                    if s % 2 == 0:
                        nc.vector.tensor_copy(
                            x1nT[:, :, s * 128:(s + 1) * 128], tp[:]
                        )
                    else:
                        nc.scalar.copy(x1nT[:, :, s * 128:(s + 1) * 128], tp[:])

                # ===== Phase F: MLP (W1 in two row-halves: the first half
                # starts right after LN2 slots 0/1 transpose, so the PE never
                # idles long enough to drop the HAM clock at the D->F seam)
                with (
                    tc.tile_pool(name="psH", bufs=2, space="PSUM") as psH,
                    tc.tile_pool(name="psO", bufs=2, space="PSUM") as psO,
                    tc.tile_pool(name="outp", bufs=2) as outp,
                ):
                    def emit_w1_half(lo, hi):
                        for nf in range(NF):
                            ps = psH.tile([128, 256], f32, tag="psH", name="psh")
                            for cb in range(NC):
                                nc.tensor.matmul(
                                    ps[:, 0:hi - lo],
                                    w1r[:, cb, nf * 128:(nf + 1) * 128],
                                    x1nT[:, cb, lo:hi],
                                    start=(cb == 0), stop=(cb == NC - 1),
                                )
                            nc.scalar.activation(
                                h1T8[:, nf // 2, nf % 2, lo:hi],
                                ps[:, 0:hi - lo],
                                AF.Gelu, bias=cF[:, 2 * HP + nf:2 * HP + nf + 1],
                            )

                    emit_transpose(0)
                    emit_transpose(1)
                    emit_w1_half(0, 256)
                    emit_transpose(2)
                    emit_transpose(3)
                    emit_w1_half(256, 512)
                    o_sb = outp.tile([128, 4, C], f32, tag="o", name="o_sb")
                    for s in range(4):
                        for g in range(2):
                            ps = psO.tile([128, 384], f32, tag="psO", name="pso")
                            for jf in range(JF):
                                nc.tensor.matmul(
                                    ps[:],
                                    h1T8[:, jf, :, s * 128:(s + 1) * 128],
                                    w2[:, jf, :, g * 384:(g + 1) * 384],
                                    start=(jf == 0), stop=False,
                                    perf_mode=PM.DoubleRow,
                                    skip_group_check=True,
                                )
                            nc.tensor.matmul(
                                ps[:], ones1[:], cB[0:1, 448 + g * 384:448 + (g + 1) * 384],
                                start=False, stop=True, skip_group_check=True,
                            )
                            nc.vector.scalar_tensor_tensor(
                                o_sb[:, s, g * 384:(g + 1) * 384], ps[:],
                                1.0 / SW,
                                x1[:, s, g * 384:(g + 1) * 384],
                                op0=ALU.mult, op1=ALU.add,
                            )
                        if s % 2 == 1:  # paired contiguous output DMAs
                            nc.sync.dma_start(
                                out_d[:, s - 1:s + 1, :],
                                o_sb[:, s - 1:s + 1, :])

    nc.compile()
    return nc


def _prep_shared(inputs):
    import ml_dtypes

    f = np.float32
    bf = ml_dtypes.bfloat16
    f8 = ml_dtypes.float8_e4m3
    g1 = np.asarray(inputs["ln1_g"], f)
    b1r = np.asarray(inputs["ln1_b"], f)
    g2 = np.asarray(inputs["ln2_g"], f)
    b2r = np.asarray(inputs["ln2_b"], f)
    Wq, Wk, Wv = (np.asarray(inputs[k], f) for k in ("Wq", "Wk", "Wv"))
    W1, W2 = np.asarray(inputs["W1"], f), np.asarray(inputs["W2"], f)

    def dr_pack(w, scale):
        # [K, M] -> [128, K/256, 2, M] with channel k = j*256 + q*128 + p
        K, M = w.shape
        return np.ascontiguousarray(
            (w * scale).reshape(K // 256, 2, 128, M).transpose(2, 0, 1, 3)
        ).astype(f8)

    def bf_pack(w):
        # [K, M] -> [128, K/128, M]
        K, M = w.shape
        return np.ascontiguousarray(
            w.reshape(K // 128, 128, M).transpose(1, 0, 2)
        ).astype(bf)

    def colmajor_bias(b, n):
        return np.ascontiguousarray(b.reshape(n, 128).T)

    rows = np.arange(128)
    import ml_dtypes as _md
    trimask = np.where(rows[:, None] > rows[None, :], -704.0, 0.0).astype(
        _md.bfloat16)
    constsf = np.hstack([
        colmajor_bias(b1r @ Wq + np.asarray(inputs["bq"], f), HP),
        colmajor_bias(b1r @ Wk + np.asarray(inputs["bk"], f), HP),
        colmajor_bias(b2r @ W1 + np.asarray(inputs["b1"], f), NF),
    ]).astype(f)
    b2pad = np.zeros((128, C), f)
    b2pad[0] = np.asarray(inputs["b2"], f)
    cb_fixed = np.hstack([
        trimask.astype(f),
        np.eye(128, dtype=f),
        np.zeros((128, NT * H), f),   # per-core vones filled in kernel()
        b2pad,
    ]).astype(_md.bfloat16)

    return {
        "constsf": np.ascontiguousarray(constsf),
        "_cb_fixed": cb_fixed,
        "wq8": dr_pack(g1[:, None] * Wq, SW),
        "wk8": dr_pack(g1[:, None] * Wk, SW),
        "wv8": dr_pack(g1[:, None] * Wv, SW),
        "w1b": bf_pack(g2[:, None] * W1),
        "w28": dr_pack(W2, SW),
        "_g1": g1, "_b1r": b1r,
        "_bv": np.asarray(inputs["bv"], f), "_Wv": Wv,
    }


def kernel(**inputs):
    import ml_dtypes
    from concourse.bass_utils import run_bass_kernel_spmd

    bf = ml_dtypes.bfloat16
    f8 = ml_dtypes.float8_e4m3

    if "nc" not in _CACHE:
        _CACHE["nc"] = _build_program()
    nc = _CACHE["nc"]

    x = np.asarray(inputs["x"], np.float64)
    shared = _prep_shared(inputs)
    g1, b1r = shared.pop("_g1"), shared.pop("_b1r")
    bv, Wv = shared.pop("_bv"), shared.pop("_Wv")

    # host LN1 (f64 stats), f32 normalized output
    mu = x.mean(-1, keepdims=True)
    var = ((x - mu) ** 2).mean(-1, keepdims=True)
    xn = ((x - mu) / np.sqrt(var + EPS)).astype(np.float32)  # [B, T, C]
    xn8 = (xn * SX).astype(f8)                               # quantized
    # residual base: ln1(x)*g1 + b1r, plus the V-bias contribution that the
    # baseline folded into the b1rb row (bv_eff enters x1 via y's V path --
    # here V biases are handled identically: bv_eff added to the base).
    bv_eff = (b1r @ Wv + bv).astype(np.float32)
    xn_base = xn * g1 + b1r + bv_eff

    def dr_pack_x(xn8_mat):
        # [Ttot, C] fp8 -> [128, JC, 2, Ttot]
        Ttot = xn8_mat.shape[0]
        return np.ascontiguousarray(
            xn8_mat.T.reshape(JC, 2, 128, Ttot).transpose(2, 0, 1, 3))

    in_maps = []
    for c8 in range(8):
        b, c = c8 // 4, c8 % 4
        pad = 3 - c
        ctx8 = np.zeros((T, C), f8)
        ctx8[pad * 128:] = xn8[b, 0:(13 + c) * 128]
        own8 = np.ascontiguousarray(
            xn8[b].reshape(16, 128, C)[c::4][::-1].reshape(R, C))  # k=3-s order
        xn_own = np.ascontiguousarray(
            xn_base[b].reshape(16, 128, C)[c::4]          # slot-major
            .transpose(1, 0, 2)).astype(np.float32)       # [128, 4, C]
        valid = np.zeros(NT, np.float32)
        valid[pad:] = 1.0
        m = dict(shared)
        cb = np.array(m.pop("_cb_fixed"))
        cb[:, 256:256 + NT * H] = np.broadcast_to(
            valid[None, :, None], (128, NT, H)).reshape(128, NT * H)
        m["constsb"] = np.ascontiguousarray(cb)
        m["xnT8"] = dr_pack_x(ctx8)
        m["wkown"] = np.ascontiguousarray(
            np.concatenate([dr_pack_x(own8), m.pop("wk8")], axis=3))
        m["xn_own"] = xn_own
        in_maps.append(m)

    trace = bool(int(os.environ.get("KERNEL_TRACE", "0")))
    try:
        res = run_bass_kernel_spmd(nc, in_maps, core_ids=list(range(8)), trace=trace)
    except ModuleNotFoundError:
        res = run_bass_kernel_spmd(nc, in_maps, core_ids=list(range(8)), trace=False)
    _CACHE["last_result"] = res

    out = np.empty((B, T, C), np.float32)
    for c8 in range(8):
        b, c = c8 // 4, c8 % 4
        for s in range(4):
            blk = c + 4 * s
            out[b, blk * 128:(blk + 1) * 128] = \
                res.results[c8]["out"][:, s, :]
    return out


# revision 54
# speedup vs baseline: 1.1889x; 1.1889x over previous
"""Trainium2 Bass kernel for a GPT-style decoder block (B=2, T=2048, C=768, H=12).

Sharding: 8 cores = 2 batches x 4 interleaved block-sets. Core c owns 128-row
blocks {c, c+4, c+8, c+12} of its batch. Its context buffer holds the 16
position-blocks [zeros x (3-c) | blocks 0..12+c]; the own blocks then sit at
the STATIC positions {3, 7, 11, 15} with causal context = position prefixes of
length {4, 8, 12, 16} blocks. Every core runs the same instruction stream.

v2 restructure (vs the LN-on-chip baseline):
- LN1 statistics, the normalize, the x^T transpose and the fp8 quantization
  all happen on the HOST (numpy) inside kernel(): the device receives
  xnT8 (pre-transposed fp8*16 context, zero-padded), xnT8own (own rows) and
  xn_own (f32 residual base = ln1(x)*g1 + b1r, own rows). This removes 96 PE
  transposes, all bn_stats/normalize work and the 30us DMA-behind-weights
  startup stall from the critical path.
- DMA order: wkown (xnT8own|wk merged, one transfer) -> packed f32 consts
  -> wq -> xnT8 -> wv -> packed bf16 consts; every separate transfer pays
  ~1.5us queue-head latency at kernel start, so small consts are packed
  into two tensors and the K-projection inputs are merged into one.
  xn_own / w2 / w1 stream later, under attention.
- Projections: K (wk stationary), Q (wq stationary, nb-inner with 4 PSUM
  banks so each DoubleRow LDWEIGHTS serves 4 N=512 matmuls), V (xnT8 tile
  stationary, j-outer g-inner so each LDWEIGHTS serves 2 matmuls).
- Attention: psa/psb double-buffered score tiles; the exp for 4 of 6 head
  pairs' first head runs on the Vector engine via a one-instruction
  Schraudolph approximation (bf16 bits of exp(x/8) == int16(A*x + B),
  uint16 output so masked negatives saturate to 0; the softmax num/den
  correlation cancels the systematic bias), balancing the exp load across
  ACT and DVE; causal tri-mask accumulated into PSUM by a PE matmul
  (tri = -704 so masked entries stay in the Schraudolph int16 window);
  per-head y/den normalization + residual accumulation run in attention's
  shadow; small warm-keeper matmuls hold the PE HAM clock at 8/8 through
  the LN2 stretch.
- MLP: W1 DMA'd into the post-attention SBUF space in 6 slice transfers
  at LN2 time (attention pools are closed by then, so the full 36KB fits);
  W1 computed in two own-row halves so the first half starts right after
  LN2 slots 0/1 transpose; W2 fp8 DoubleRow as before.

Numerics vs the fixed-seed reference: ~1.75e-2 measured on HW (gate 2e-2):
fp8e4 QKV & W2 (DoubleRow), bf16 scores/P/V and W1, fp32 LN statistics
(host, f64), softmax normalization, residuals and output.
"""

import os

import numpy as np

B, T, C = 2, 2048, 768
H, DH = 12, 64
F = 4 * C
R = 512            # own rows per core
NT = 16            # ctx position blocks
NC = C // 128      # 6
JC = NC // 2       # 3 channel pairs
NF = F // 128      # 24
JF = NF // 2       # 12 hidden pairs
HP = H // 2        # 6 head pairs
VS = 66            # per-head stride in Vt (64 v + 1 ones + pad)
EPS = 1e-3
SX = 16.0          # fp8 scale on normalized activations
SW = 512.0         # fp8 scale on weights
SXW = SX * SW      # 8192

_CACHE = {}


def _build_program():
    import concourse.bass as bass  # noqa: F401
    import concourse.mybir as mybir
    import concourse.tile as tile
    from concourse import bacc

    dt = mybir.dt
    f32 = dt.float32
    bf16 = dt.bfloat16
    fp8 = dt.float8e4
    AF = mybir.ActivationFunctionType
    ALU = mybir.AluOpType
    PM = mybir.MatmulPerfMode

    nc = bacc.Bacc("TRN2", target_bir_lowering=False, debug=False, num_devices=8)

    # ---- DRAM I/O ----
    # all small consts packed into two tensors: each separate DMA pays
    # ~1.5us of queue-head latency at kernel start, so 7 transfers -> 2.
    # constsf f32 [128, 36]: bq | bk | b1 (column-major per-hp/nf biases)
    # constsb bf16 [128, 1216]: tri | ident | vones(16*12) | b2row(row 0)
    cf_d = nc.dram_tensor("constsf", [128, 2 * HP + NF], f32,
                          kind="ExternalInput")
    cb_d = nc.dram_tensor("constsb", [128, 256 + NT * H + C], bf16,
                          kind="ExternalInput")
    # xnT8own and wk merged into one tensor/DMA (queue-head latency):
    # cols 0:R = own rows, R:R+C = wk
    ko_d = nc.dram_tensor("wkown", [128, JC, 2, R + C], fp8,
                          kind="ExternalInput")
    xt8_d = nc.dram_tensor("xnT8", [128, JC, 2, T], fp8, kind="ExternalInput")
    wq_d = nc.dram_tensor("wq8", [128, JC, 2, C], fp8, kind="ExternalInput")
    wv_d = nc.dram_tensor("wv8", [128, JC, 2, C], fp8, kind="ExternalInput")
    xn_own_d = nc.dram_tensor("xn_own", [128, 4, C], f32, kind="ExternalInput")
    w2_d = nc.dram_tensor("w28", [128, JF, 2, C], fp8, kind="ExternalInput")
    w1_d = nc.dram_tensor("w1b", [128, NC, F], bf16, kind="ExternalInput")
    out_d = nc.dram_tensor("out", [R, C], f32, kind="ExternalOutput")

    with tile.TileContext(nc) as tc:
        with (
            tc.tile_pool(name="const", bufs=1) as constp,
            tc.tile_pool(name="keep", bufs=1) as keepp,
            tc.tile_pool(name="w2pool", bufs=1) as w2p,
            tc.tile_pool(name="stat2", bufs=1) as stat2p,
        ):
            cF = constp.tile([128, 2 * HP + NF], f32)
            cB = constp.tile([128, 256 + NT * H + C], bf16)
            ones1 = constp.tile([1, 128], bf16)
            nc.vector.memset(ones1[:], 1.0)
            eps_t = constp.tile([128, 1], f32)
            nc.vector.memset(eps_t[:], EPS)

            x1 = keepp.tile([128, 4, C], f32)        # residual base, then +y
            yrow = keepp.tile([128, 4, C], f32)      # y/den, token-major
            y_sb = keepp.tile([128, 4, H, 65], bf16)  # y token-major; k=3-s

            with (
                tc.tile_pool(name="xnT8", bufs=1) as xnT8p,
                tc.tile_pool(name="QT", bufs=1) as QTp,
                tc.tile_pool(name="KT", bufs=1) as KTp,
                tc.tile_pool(name="V", bufs=1) as Vp,
            ):
                wko = KTp.tile([128, JC, 2, R + C], fp8)
                nc.sync.dma_start(wko[:], ko_d[:])
                KT = KTp.tile([128, HP, R], bf16)
                QT = QTp.tile([128, HP, T], bf16)
                Vt = Vp.tile([128, NT, H, VS], bf16)
                xnT8 = xnT8p.tile([128, JC, 2, T], fp8)

                # ===== Phase A: QKV projections =====
                with (
                    tc.tile_pool(name="wqkv", bufs=1) as wp,
                    tc.tile_pool(name="psQ", bufs=1, space="PSUM") as psQ,
                    tc.tile_pool(name="psV", bufs=2, space="PSUM") as psV,
                ):
                    nc.sync.dma_start(cF[:], cf_d[:])
                    wq = wp.tile([128, JC, 2, C], fp8, name="wq8")
                    nc.sync.dma_start(wq[:], wq_d[:])
                    nc.sync.dma_start(xnT8[:], xt8_d[:])
                    wv = wp.tile([128, JC, 2, C], fp8, name="wv8")
                    nc.sync.dma_start(wv[:], wv_d[:])
                    nc.sync.dma_start(cB[:], cb_d[:])
                    # ones column of Vt (zero for padding blocks)
                    nc.vector.tensor_copy(
                        Vt[:, :, :, 64],
                        cB[:, 256:256 + NT * H].rearrange(
                            "p (t h) -> p t h", t=NT),
                    )

                    # K projection (own rows): wk stationary per (hp, j)
                    for hp in range(HP):
                        ps = psQ.tile([128, 512], f32, tag="q%d" % (hp % 4),
                                      name="psk")
                        for j in range(JC):
                            nc.tensor.matmul(
                                ps[:],
                                wko[:, j, :, R + hp * 128:R + (hp + 1) * 128],
                                wko[:, j, :, 0:R],
                                start=(j == 0), stop=(j == JC - 1),
                                perf_mode=PM.DoubleRow,
                            )
                        if hp % 2 == 0:
                            nc.vector.tensor_scalar(
                                KT[:, hp, :], ps[:], 1.0 / SXW, cF[:, HP + hp:HP + hp + 1],
                                op0=ALU.mult, op1=ALU.add,
                            )
                        else:
                            nc.scalar.activation(
                                KT[:, hp, :], ps[:], AF.Identity,
                                bias=cF[:, HP + hp:HP + hp + 1], scale=1.0 / SXW,
                            )

                    # Q projection: wq stationary per (hp, j) serves 4 matmuls
                    for hp in range(HP):
                        pss = [psQ.tile([128, 512], f32, tag="q%d" % nb,
                                        name="psq%d" % nb) for nb in range(4)]
                        for j in range(JC):
                            for nb in range(4):
                                nc.tensor.matmul(
                                    pss[nb][:],
                                    wq[:, j, :, hp * 128:(hp + 1) * 128],
                                    xnT8[:, j, :, nb * 512:(nb + 1) * 512],
                                    start=(j == 0), stop=(j == JC - 1),
                                    perf_mode=PM.DoubleRow,
                                )
                        for nb in range(4):
                            if (hp + nb) % 2 == 0:
                                nc.scalar.activation(
                                    QT[:, hp, nb * 512:(nb + 1) * 512],
                                    pss[nb][:], AF.Identity,
                                    bias=cF[:, hp:hp + 1], scale=1.0 / SXW,
                                )
                            else:
                                nc.vector.tensor_scalar(
                                    QT[:, hp, nb * 512:(nb + 1) * 512],
                                    pss[nb][:], 1.0 / SXW, cF[:, hp:hp + 1],
                                    op0=ALU.mult, op1=ALU.add,
                                )


                    # V projection: xnT8 tile stationary per (tb, j)
                    for tb in range(NT):
                        ps = psV.tile([128, 2, 8, 64], f32, tag="psV", name="psv")
                        for j in range(JC):
                            for g in range(2):
                                nc.tensor.matmul(
                                    ps[:, g, 0:6, :],
                                    xnT8[:, j, :, tb * 128:(tb + 1) * 128],
                                    wv[:, j, :, g * 384:(g + 1) * 384],
                                    start=(j == 0), stop=(j == JC - 1),
                                    perf_mode=PM.DoubleRow,
                                )
                        for g in range(2):
                            if (tb + g) % 2 == 0:
                                nc.vector.tensor_scalar(
                                    Vt[:, tb, g * 6:(g + 1) * 6, 0:64],
                                    ps[:, g, 0:6, :], 1.0 / SXW, None,
                                    op0=ALU.mult,
                                )
                            else:
                                nc.scalar.activation(
                                    Vt[:, tb, g * 6:(g + 1) * 6, 0:64],
                                    ps[:, g, 0:6, :],
                                    AF.Identity, scale=1.0 / SXW,
                                )

                # residual base + deferred weights (DMA after the QKV weights)
                nc.sync.dma_start(x1[:], xn_own_d[:])
                w2 = w2p.tile([128, JF, 2, C], fp8, name="w28")
                nc.sync.dma_start(w2[:], w2_d[:])

                # ===== Phase C: attention =====
                # (st6 lives here so the g=0 stats can run in C's shadow)
                # Schraudolph exp on DVE for head A: the bf16 bit pattern of
                # exp(x/8) == int16(AEXP*x + BEXP) (the /65536 folds the
                # >>16 into the mac; the int16 convert rounds, ~0.8% noise
                # inside Schraudolph's 3.5% band; num/den correlation cancels
                # the systematic bias in the softmax). tri = -704 ~=
                # -B/A makes masked entries land near zero, so the int16 is
                # a ~2^-117 bf16 denormal (effectively zero) without a
                # clamp, while exp(-88) == 0 on the ACT path too.
                AEXP = float(2.0 ** 23 / np.log(2.0) * 0.125 / 65536.0)
                BEXP = float((127 * 2 ** 23 - 366000) / 65536.0)
                u16 = dt.uint16  # f32->uint16 saturates negatives to 0
                with (
                    tc.tile_pool(name="exps", bufs=2) as expp,
                    tc.tile_pool(name="yT", bufs=2) as ytp,
                    tc.tile_pool(name="yn", bufs=2) as ynp,
                    tc.tile_pool(name="psS", bufs=2, space="PSUM") as psS,
                    tc.tile_pool(name="psY", bufs=2, space="PSUM") as psY,
                ):
                    def emit_scores_pair(hp, expAB):
                        # one step per ctx pair: psa = head 2hp, psb = head
                        # 2hp+1; head A exp on DVE, head B exp on ACT
                        for jp in range(NT // 2):
                            Np = (4 - jp // 2) * 128
                            diag = (jp % 2 == 1)  # P=2jp+1 is a diag block
                            psa = psS.tile([128, 2, 512], f32, tag="psS",
                                           name="pssa")
                            psb = psS.tile([128, 2, 512], f32, tag="psS",
                                           name="pssb")
                            for ql in range(2):
                                P = 2 * jp + ql
                                dq = diag and ql == 1
                                for z, ps in ((0, psa), (1, psb)):
                                    nc.tensor.matmul(
                                        ps[:, ql, 0:Np],
                                        QT[64 * z:64 * z + 64, hp,
                                           P * 128:(P + 1) * 128],
                                        KT[64 * z:64 * z + 64, hp, 0:Np],
                                        start=True, stop=not dq,
                                        skip_group_check=dq,
                                    )
                            if diag:  # accumulate tri into the diag slice
                                for ps in (psa, psb):
                                    nc.tensor.matmul(
                                        ps[:, 1, Np - 128:Np],
                                        cB[:, 128:256], cB[:, 0:128],
                                        start=False, stop=True,
                                        skip_group_check=True,
                                    )
                            if hp != 1 and hp != 4:
                                nc.vector.tensor_scalar(
                                    expAB[:, 0, 2 * jp:2 * jp + 2, 0:Np].bitcast(u16),
                                    psa[:, :, 0:Np], AEXP, BEXP,
                                    op0=ALU.mult, op1=ALU.add,
                                )
                            else:
                                nc.scalar.activation(
                                    expAB[:, 0, 2 * jp:2 * jp + 2, 0:Np],
                                    psa[:, :, 0:Np], AF.Exp, scale=0.125,
                                )
                            nc.scalar.activation(
                                expAB[:, 1, 2 * jp:2 * jp + 2, 0:Np],
                                psb[:, :, 0:Np], AF.Exp, scale=0.125,
                            )
                            yield

                    def emit_pv(h, z, expAB):
                        # generator: one step per ctx pair (2 PV matmuls)
                        psy = psY.tile([128, 512], f32, tag="psY", name="psy")
                        for jp in range(NT // 2):
                            for ql in range(2):
                                P = 2 * jp + ql
                                Np = (4 - P // 4) * 128
                                nc.tensor.matmul(
                                    psy[0:65, 0:Np],
                                    Vt[:, P, h, 0:65],
                                    expAB[:, z, P, 0:Np],
                                    start=(P == 0), stop=(P == NT - 1),
                                    skip_group_check=True,
                                )
                            yield
                        yTb = ytp.tile([128, 512], bf16, tag="yT", name="yT")
                        nc.vector.tensor_copy(yTb[0:65, :], psy[0:65, :])
                        tpy = psY.tile([128, 4, 66], bf16, tag="psTy", name="tpy")
                        for k in range(4):
                            nc.tensor.matmul(
                                tpy[:, k, 0:65], yTb[0:65, k * 128:(k + 1) * 128],
                                cB[0:65, 128:193], is_transpose=True,
                                start=True, stop=True,
                            )
                        if h % 2 == 0:
                            nc.scalar.copy(y_sb[:, :, h, :], tpy[:, :, 0:65])
                        else:
                            nc.vector.tensor_copy(y_sb[:, :, h, :], tpy[:, :, 0:65])
                        yield
                        # y/den into yrow (runs in attention's shadow)
                        den = ynp.tile([128, 4], f32, tag="den", name="den")
                        nc.vector.tensor_copy(den[:], y_sb[:, :, h, 64])
                        rec = ynp.tile([128, 4], f32, tag="rec", name="rec")
                        nc.vector.reciprocal(rec[:], den[:])
                        for s in range(4):
                            k = 3 - s
                            if (h + s) % 2 == 0:
                                nc.vector.tensor_scalar(
                                    yrow[:, s, h * 64:(h + 1) * 64],
                                    y_sb[:, k, h, 0:64],
                                    rec[:, k:k + 1], None, op0=ALU.mult,
                                )
                            else:
                                nc.scalar.activation(
                                    yrow[:, s, h * 64:(h + 1) * 64],
                                    y_sb[:, k, h, 0:64],
                                    AF.Identity, scale=rec[:, k:k + 1],
                                )
                        yield

                    # software pipeline: pair i scores/exp woven with the
                    # previous pair's two PV streams
                    prev_pvs = []
                    for hp2 in range(HP):
                        expAB = expp.tile([128, 2, NT, 512], bf16,
                                          tag="expST", name="expAB")
                        sc = emit_scores_pair(hp2, expAB)
                        for _ in sc:
                            for pv in prev_pvs:
                                next(pv, None)
                        for pv in prev_pvs:  # drain tails
                            for _ in pv:
                                pass
                        prev_pvs = [emit_pv(2 * hp2, 0, expAB),
                                    emit_pv(2 * hp2 + 1, 1, expAB)]
                    # heads 0-9 are final: fold them into the residual and
                    # take the g=0 LN2 stats now, then drain the last PV
                    # pair -- its matmuls keep the PE HAM window alive while
                    # this DVE-serial stretch runs, so the MLP starts warm
                    st6s = [stat2p.tile([128, 3, 6], f32, tag="st6%d" % s,
                                        name="st6b") for s in range(4)]
                    for s in range(4):
                        nc.vector.tensor_add(
                            x1[:, s, 0:384], x1[:, s, 0:384], yrow[:, s, 0:384])
                        nc.vector.tensor_add(
                            x1[:, s, 384:640], x1[:, s, 384:640],
                            yrow[:, s, 384:640])
                    for s in range(4):
                        nc.vector.bn_stats(st6s[s][:, 0, :], x1[:, s, 0:384])
                        nc.vector.bn_stats(st6s[s][:, 1, :], x1[:, s, 384:640])
                    for pv in prev_pvs:
                        for _ in pv:
                            pass

            # ===== Phase D: +y, LN2 =====
            with (
                tc.tile_pool(name="x1nT", bufs=1) as x1nTp,
                tc.tile_pool(name="h1T8", bufs=1) as h1p,
                tc.tile_pool(name="w1res", bufs=1) as w1rp,
                tc.tile_pool(name="x1nbf", bufs=1) as x1nbfp,
                tc.tile_pool(name="psT2", bufs=2, space="PSUM") as psT2,
            ):
                x1nT = x1nTp.tile([128, NC, R], bf16)
                h1T8 = h1p.tile([128, JF, 2, R], fp8)
                w1r = w1rp.tile([128, NC, F], bf16, name="w1b")
                for i in range(6):
                    nc.sync.dma_start(w1r[:, :, i * 512:(i + 1) * 512],
                                      w1_d[:, :, i * 512:(i + 1) * 512])

                warm = psT2.tile([128, 128], f32, tag="warm", name="warm")
                st2s, rstds, nmbs, x1ns = [], [], [], []
                for s in range(4):
                    nc.vector.tensor_add(
                        x1[:, s, 640:768], x1[:, s, 640:768],
                        yrow[:, s, 640:768])
                    nc.vector.bn_stats(st6s[s][:, 2, :], x1[:, s, 640:768])
                for s in range(4):
                    st2 = stat2p.tile([128, 2], f32, tag="st2%d" % s, name="st2b")
                    nc.vector.bn_aggr(st2[:], st6s[s][:])
                    st2s.append(st2)
                    std = stat2p.tile([128, 1], f32, tag="std%d" % s, name="stdb")
                    nc.scalar.activation(std[:], st2[:, 1:2], AF.Sqrt,
                                         bias=eps_t[:])
                    rstd = stat2p.tile([128, 1], f32, tag="rstd%d" % s,
                                       name="rstdb")
                    nc.vector.reciprocal(rstd[:], std[:])
                    rstds.append(rstd)
                for s in range(4):
                    nc.tensor.matmul(
                        warm[0:2, 0:128], st2s[s][:], x1[:, s, 0:128],
                        start=True, stop=True, skip_group_check=True,
                    )
                    nmb = stat2p.tile([128, 1], f32, tag="nmb%d" % s, name="nmbb")
                    nc.vector.tensor_scalar(
                        nmb[:], st2s[s][:, 0:1], rstds[s][:], -1.0,
                        op0=ALU.mult, op1=ALU.mult,
                    )
                    x1n = x1nbfp.tile([128, C], bf16, tag="x1n%d" % s, name="x1n")
                    nc.scalar.activation(
                        x1n[:], x1[:, s, :], AF.Identity, bias=nmb[:],
                        scale=rstds[s][:]
                    )
                    x1ns.append(x1n)
                def emit_transpose(s):
                    tp = psT2.tile([128, NC, 128], bf16, tag="psT2", name="tpb")
                    for cb in range(NC):
                        nc.tensor.matmul(
                            tp[:, cb, :],
                            x1ns[s][:, cb * 128:(cb + 1) * 128],
                            cB[:, 128:256], is_transpose=True, start=True, stop=True,
                        )
                    if s % 2 == 0:
                        nc.vector.tensor_copy(
                            x1nT[:, :, s * 128:(s + 1) * 128], tp[:]
                        )
                    else:
                        nc.scalar.copy(x1nT[:, :, s * 128:(s + 1) * 128], tp[:])

                # ===== Phase F: MLP (W1 in two row-halves: the first half
                # starts right after LN2 slots 0/1 transpose, so the PE never
                # idles long enough to drop the HAM clock at the D->F seam)
                with (
                    tc.tile_pool(name="psH", bufs=2, space="PSUM") as psH,
                    tc.tile_pool(name="psO", bufs=2, space="PSUM") as psO,
                    tc.tile_pool(name="outp", bufs=2) as outp,
                ):
                    def emit_w1_half(lo, hi):
                        for nf in range(NF):
                            ps = psH.tile([128, 256], f32, tag="psH", name="psh")
                            for cb in range(NC):
                                nc.tensor.matmul(
                                    ps[:, 0:hi - lo],
                                    w1r[:, cb, nf * 128:(nf + 1) * 128],
                                    x1nT[:, cb, lo:hi],
                                    start=(cb == 0), stop=(cb == NC - 1),
                                )
                            nc.scalar.activation(
                                h1T8[:, nf // 2, nf % 2, lo:hi],
                                ps[:, 0:hi - lo],
                                AF.Gelu, bias=cF[:, 2 * HP + nf:2 * HP + nf + 1],
                            )

                    emit_transpose(0)
                    emit_transpose(1)
                    emit_w1_half(0, 256)
                    emit_transpose(2)
                    emit_transpose(3)
                    emit_w1_half(256, 512)
                    for s in range(4):
                        o_sb = outp.tile([128, C], f32, tag="o", name="o_sb")
                        for g in range(2):
                            ps = psO.tile([128, 384], f32, tag="psO", name="pso")
                            for jf in range(JF):
                                nc.tensor.matmul(
                                    ps[:],
                                    h1T8[:, jf, :, s * 128:(s + 1) * 128],
                                    w2[:, jf, :, g * 384:(g + 1) * 384],
                                    start=(jf == 0), stop=False,
                                    perf_mode=PM.DoubleRow,
                                    skip_group_check=True,
                                )
                            nc.tensor.matmul(
                                ps[:], ones1[:], cB[0:1, 448 + g * 384:448 + (g + 1) * 384],
                                start=False, stop=True, skip_group_check=True,
                            )
                            nc.vector.scalar_tensor_tensor(
                                o_sb[:, g * 384:(g + 1) * 384], ps[:], 1.0 / SW,
                                x1[:, s, g * 384:(g + 1) * 384],
                                op0=ALU.mult, op1=ALU.add,
                            )
                        nc.sync.dma_start(out_d[s * 128:(s + 1) * 128, :], o_sb[:])

    nc.compile()
    return nc


def _prep_shared(inputs):
    import ml_dtypes

    f = np.float32
    bf = ml_dtypes.bfloat16
    f8 = ml_dtypes.float8_e4m3
    g1 = np.asarray(inputs["ln1_g"], f)
    b1r = np.asarray(inputs["ln1_b"], f)
    g2 = np.asarray(inputs["ln2_g"], f)
    b2r = np.asarray(inputs["ln2_b"], f)
    Wq, Wk, Wv = (np.asarray(inputs[k], f) for k in ("Wq", "Wk", "Wv"))
    W1, W2 = np.asarray(inputs["W1"], f), np.asarray(inputs["W2"], f)

    def dr_pack(w, scale):
        # [K, M] -> [128, K/256, 2, M] with channel k = j*256 + q*128 + p
        K, M = w.shape
        return np.ascontiguousarray(
            (w * scale).reshape(K // 256, 2, 128, M).transpose(2, 0, 1, 3)
        ).astype(f8)

    def bf_pack(w):
        # [K, M] -> [128, K/128, M]
        K, M = w.shape
        return np.ascontiguousarray(
            w.reshape(K // 128, 128, M).transpose(1, 0, 2)
        ).astype(bf)

    def colmajor_bias(b, n):
        return np.ascontiguousarray(b.reshape(n, 128).T)

    rows = np.arange(128)
    import ml_dtypes as _md
    trimask = np.where(rows[:, None] > rows[None, :], -704.0, 0.0).astype(
        _md.bfloat16)
    constsf = np.hstack([
        colmajor_bias(b1r @ Wq + np.asarray(inputs["bq"], f), HP),
        colmajor_bias(b1r @ Wk + np.asarray(inputs["bk"], f), HP),
        colmajor_bias(b2r @ W1 + np.asarray(inputs["b1"], f), NF),
    ]).astype(f)
    b2pad = np.zeros((128, C), f)
    b2pad[0] = np.asarray(inputs["b2"], f)
    cb_fixed = np.hstack([
        trimask.astype(f),
        np.eye(128, dtype=f),
        np.zeros((128, NT * H), f),   # per-core vones filled in kernel()
        b2pad,
    ]).astype(_md.bfloat16)

    return {
        "constsf": np.ascontiguousarray(constsf),
        "_cb_fixed": cb_fixed,
        "wq8": dr_pack(g1[:, None] * Wq, SW),
        "wk8": dr_pack(g1[:, None] * Wk, SW),
        "wv8": dr_pack(g1[:, None] * Wv, SW),
        "w1b": bf_pack(g2[:, None] * W1),
        "w28": dr_pack(W2, SW),
        "_g1": g1, "_b1r": b1r,
        "_bv": np.asarray(inputs["bv"], f), "_Wv": Wv,
    }


def kernel(**inputs):
    import ml_dtypes
    from concourse.bass_utils import run_bass_kernel_spmd

    bf = ml_dtypes.bfloat16
    f8 = ml_dtypes.float8_e4m3

    if "nc" not in _CACHE:
        _CACHE["nc"] = _build_program()
    nc = _CACHE["nc"]

    x = np.asarray(inputs["x"], np.float64)
    shared = _prep_shared(inputs)
    g1, b1r = shared.pop("_g1"), shared.pop("_b1r")
    bv, Wv = shared.pop("_bv"), shared.pop("_Wv")

    # host LN1 (f64 stats), f32 normalized output
    mu = x.mean(-1, keepdims=True)
    var = ((x - mu) ** 2).mean(-1, keepdims=True)
    xn = ((x - mu) / np.sqrt(var + EPS)).astype(np.float32)  # [B, T, C]
    xn8 = (xn * SX).astype(f8)                               # quantized
    # residual base: ln1(x)*g1 + b1r, plus the V-bias contribution that the
    # baseline folded into the b1rb row (bv_eff enters x1 via y's V path --
    # here V biases are handled identically: bv_eff added to the base).
    bv_eff = (b1r @ Wv + bv).astype(np.float32)
    xn_base = xn * g1 + b1r + bv_eff

    def dr_pack_x(xn8_mat):
        # [Ttot, C] fp8 -> [128, JC, 2, Ttot]
        Ttot = xn8_mat.shape[0]
        return np.ascontiguousarray(
            xn8_mat.T.reshape(JC, 2, 128, Ttot).transpose(2, 0, 1, 3))

    in_maps = []
    for c8 in range(8):
        b, c = c8 // 4, c8 % 4
        pad = 3 - c
        ctx8 = np.zeros((T, C), f8)
        ctx8[pad * 128:] = xn8[b, 0:(13 + c) * 128]
        own8 = np.ascontiguousarray(
            xn8[b].reshape(16, 128, C)[c::4][::-1].reshape(R, C))  # k=3-s order
        xn_own = np.ascontiguousarray(
            xn_base[b].reshape(16, 128, C)[c::4]          # slot-major
            .transpose(1, 0, 2)).astype(np.float32)       # [128, 4, C]
        valid = np.zeros(NT, np.float32)
        valid[pad:] = 1.0
        m = dict(shared)
        cb = np.array(m.pop("_cb_fixed"))
        cb[:, 256:256 + NT * H] = np.broadcast_to(
            valid[None, :, None], (128, NT, H)).reshape(128, NT * H)
        m["constsb"] = np.ascontiguousarray(cb)
        m["xnT8"] = dr_pack_x(ctx8)
        m["wkown"] = np.ascontiguousarray(
            np.concatenate([dr_pack_x(own8), m.pop("wk8")], axis=3))
        m["xn_own"] = xn_own
        in_maps.append(m)

    trace = bool(int(os.environ.get("KERNEL_TRACE", "0")))
    try:
        res = run_bass_kernel_spmd(nc, in_maps, core_ids=list(range(8)), trace=trace)
    except ModuleNotFoundError:
        res = run_bass_kernel_spmd(nc, in_maps, core_ids=list(range(8)), trace=False)
    _CACHE["last_result"] = res

    out = np.empty((B, T, C), np.float32)
    for c8 in range(8):
        b, c = c8 // 4, c8 % 4
        for s in range(4):
            blk = c + 4 * s
            out[b, blk * 128:(blk + 1) * 128] = \
                res.results[c8]["out"][s * 128:(s + 1) * 128]
    return out
